# revision 8
# baseline (speedup 1.0000x reference)
"""Trainium2 Bass kernel for nn_MultiHeadAttention (B=2, S=2048, DM=1024, H=8).

Sharding: data-parallel on batch x tensor-parallel on heads.
Core c in 0..7 handles batch b = c//4 and heads {2*(c%4), 2*(c%4)+1}.
Each core computes its two heads' full attention and the partial
out-projection (a 1024x2048 partial sum); the host adds the 4 partials
per batch and transposes back to (S, DOUT).

Device dataflow is feature-major ("transposed") throughout:
  - host ships q/k/v transposed to (DM, S) per batch (bf16)
  - QpT/KpT (d, S) = W_h^T @ xT via matmuls with W stationary
  - Vp natural (S, d) computed directly with xT tiles stationary
  - S^T pair-tile (128 keys x 1024 queries, 2 PSUM banks) -> one ACT exp
    (scale=1/sqrt(D)) -> one DVE mask multiply (bf16 2x mode)
  - out^T accumulated over key chunks; rowsums via ones-column matmuls
    bank-packed at partitions 0/32 of one PSUM bank
  - early PSUM evacuation, reciprocal on a [128,8]-reshaped layout,
    DMA-broadcast, TT normalize
  - out-projection stays feature-major; bo added during DVE evacuation
"""

import sys

sys.path.insert(0, "/opt/trn_rl_repo")

import numpy as np
import ml_dtypes

import concourse.bass as bass
import concourse.tile as tile
from concourse import bacc, mybir
from concourse.bass import ts, ds
from concourse.bass_utils import run_bass_kernel_spmd

BF16 = mybir.dt.bfloat16
F32 = mybir.dt.float32
Copy = mybir.ActivationFunctionType.Copy
Exp = mybir.ActivationFunctionType.Exp
ADD = mybir.AluOpType.add
MULT = mybir.AluOpType.mult

B, S, DM, H, DOUT = 2, 2048, 1024, 8, 1024
D = DM // H            # 128 head dim
NH = 2                 # heads per core
KC = DM // 128         # 8 contraction chunks for projections
OC = S // 128          # 16 key chunks
NT = 512               # PSUM-bank-sized free tile (fp32)
IT = S // NT           # 4 query tiles
SCALE = float(1.0 / np.sqrt(np.float32(D)))


def build():
    nc = bacc.Bacc(None, target_bir_lowering=False)

    xT = nc.dram_tensor("xT", [3, DM, S], BF16, kind="ExternalInput")
    maskT = nc.dram_tensor("maskT", [S, S], BF16, kind="ExternalInput")
    w_qkv = nc.dram_tensor("w_qkv", [128, 3, KC, NH, D], BF16, kind="ExternalInput")
    b_qkv = nc.dram_tensor("b_qkv", [128, 3, NH], F32, kind="ExternalInput")
    bv_in = nc.dram_tensor("bv", [1, NH, D], BF16, kind="ExternalInput")
    wo = nc.dram_tensor("wo", [D, NH, DOUT], BF16, kind="ExternalInput")
    bo = nc.dram_tensor("bo", [128, DOUT // 128], F32, kind="ExternalInput")
    outT = nc.dram_tensor("outT", [DOUT, S], F32, kind="ExternalOutput")

    dma_q = [nc.sync, nc.scalar, nc.gpsimd]  # spread big loads over queues

    with tile.TileContext(nc) as tc:
        with (
            tc.tile_pool(name="const", bufs=1) as constp,
            tc.tile_pool(name="xin", bufs=9) as xp,
            tc.tile_pool(name="ptile", bufs=3) as pp,
            tc.tile_pool(name="rb", bufs=2) as rbp,
            tc.tile_pool(name="fout", bufs=4) as fop,
            tc.tile_pool(name="psum", bufs=4, space="PSUM") as psp,
            tc.tile_pool(name="dram", bufs=2, space="DRAM") as dramp,
        ):
            # ---- constants (small, load first) ----
            b_sb = constp.tile([128, 3, NH], F32)  # per-partition-d biases q/k
            nc.sync.dma_start(out=b_sb, in_=b_qkv[:])
            bv_sb = constp.tile([1, NH, D], BF16)
            nc.scalar.dma_start(out=bv_sb, in_=bv_in[:])
            bo_sb = constp.tile([128, DOUT // 128], F32)
            nc.gpsimd.dma_start(out=bo_sb, in_=bo[:])
            ones_row = constp.tile([1, NT], BF16)
            nc.vector.memset(ones_row, 1.0)
            ones_col = constp.tile([128, 1], BF16)
            nc.vector.memset(ones_col, 1.0)

            # ---- weight chunks + x tiles interleaved in need order ----
            w_sb = constp.tile([128, 3, KC, NH, D], BF16)
            wo_sb = constp.tile([D, NH, DOUT], BF16)
            xts_all = []
            for t in range(3):
                row = []
                for k in range(KC):
                    qi = t * KC + k
                    nc.sync.dma_start(
                        out=w_sb[:, t, k, :, :], in_=w_qkv[:, t, k, :, :]
                    )
                    xt = xp.tile([128, S], BF16, tag="x", name=f"x{t}_{k}")
                    dma_q[qi % 3].dma_start(
                        out=xt, in_=xT[t, k * 128 : (k + 1) * 128, :]
                    )
                    row.append(xt)
                xts_all.append(row)

            # mask after x (needed only from attention onward), all queues
            mask_sb = constp.tile([128, OC, S], BF16)
            for oc in range(OC):
                dma_q[oc % 3].dma_start(
                    out=mask_sb[:, oc, :], in_=maskT[oc * 128 : (oc + 1) * 128, :]
                )

            nc.scalar.dma_start(out=wo_sb, in_=wo[:])

            # ---- Q/K projections: qk_sb[d, t, h, s]; bias added on DVE evac ----
            qk_sb = constp.tile([128, 2, NH, S], BF16)
            for t in range(2):
                xts = xts_all[t]
                for h in range(NH):
                    acc = [
                        psp.tile([128, NT], F32, tag="acc", name=f"acc{it}")
                        for it in range(IT)
                    ]
                    for k in range(KC):
                        for it in range(IT):
                            nc.tensor.matmul(
                                acc[it],
                                w_sb[:, t, k, h, :],
                                xts[k][:, ts(it, NT)],
                                start=(k == 0),
                                stop=(k == KC - 1),
                            )
                    for it in range(IT):
                        nc.vector.tensor_scalar_add(
                            out=qk_sb[:, t, h, ts(it, NT)],
                            in0=acc[it],
                            scalar1=b_sb[:, t, h : h + 1],
                        )

            # ---- V projection, natural layout: vp_sb[s%128, oc, h, d] ----
            vp_sb = constp.tile([128, OC, NH, D], BF16)
            xts = xts_all[2]
            for oc in range(OC):
                vacc = psp.tile([128, NH * D], F32, tag="acc")
                nc.tensor.matmul(
                    vacc, ones_row[:, 0:128], bv_sb[:, :, :], start=True, stop=False
                )
                for k in range(KC):
                    nc.tensor.matmul(
                        vacc,
                        xts[k][:, ts(oc, 128)],
                        w_sb[:, 2, k, :, :],
                        start=False,
                        stop=(k == KC - 1),
                    )
                nc.scalar.activation(out=vp_sb[:, oc, :, :], in_=vacc, func=Copy)

            # ---- attention per head / per half (1024 queries) ----
            outn_sb = constp.tile([128, NH, S], BF16)
            for half in range(2):
                for h in range(NH):
                    i0 = half * 2 * NT  # start query of this half
                    outp = [
                        psp.tile([128, NT], F32, tag="acc", name=f"outp{j}")
                        for j in range(2)
                    ]
                    rp = psp.tile([128, NT], F32, tag="acc", name="rp")
                    for oc in range(OC):
                        sps = psp.tile([128, 2 * NT], F32, tag="s", bufs=2)
                        for j in range(2):
                            nc.tensor.matmul(
                                sps[:, ts(j, NT)],
                                qk_sb[:, 1, h, ds(oc * 128, 128)],
                                qk_sb[:, 0, h, ds(i0 + j * NT, NT)],
                                start=True,
                                stop=True,
                            )
                        p = pp.tile([128, 2 * NT], BF16, tag="p")
                        nc.scalar.activation(
                            out=p, in_=sps, func=Exp, bias=0.0, scale=SCALE
                        )
                        pm = pp.tile([128, 2 * NT], BF16, tag="pm")
                        nc.vector.tensor_mul(
                            pm, p, mask_sb[:, oc, ds(i0, 2 * NT)]
                        )
                        for j in range(2):
                            nc.tensor.matmul(
                                outp[j],
                                vp_sb[:, oc, h, :],
                                pm[:, ts(j, NT)],
                                start=(oc == 0),
                                stop=(oc == OC - 1),
                            )
                            nc.tensor.matmul(
                                rp[32 * j : 32 * j + 1, :],
                                ones_col,
                                pm[:, ts(j, NT)],
                                start=(oc == 0),
                                stop=(oc == OC - 1),
                            )
                    # early evacuation frees the PSUM banks for the next half
                    osb = rbp.tile([128, 2 * NT], F32, tag="osb")
                    for j in range(2):
                        nc.vector.tensor_copy(osb[:, ts(j, NT)], outp[j])
                    r2 = rbp.tile([33, NT], F32, tag="r2")
                    for j in range(2):
                        nc.vector.tensor_copy(
                            r2[32 * j : 32 * j + 1, :], rp[32 * j : 32 * j + 1, :]
                        )
                    # reshape rowsums through DRAM so reciprocal uses 128 lanes
                    rd = dramp.tile([2, NT], F32, tag="rd")
                    for j in range(2):
                        nc.sync.dma_start(
                            out=rd[j : j + 1, :], in_=r2[32 * j : 32 * j + 1, :]
                        )
                    rseg = rbp.tile([128, 8], F32, tag="rseg")
                    nc.sync.dma_start(
                        out=rseg,
                        in_=rd[:].rearrange("a b -> (a b)").rearrange(
                            "(p j) -> p j", p=128
                        ),
                    )
                    nc.vector.reciprocal(rseg, rseg)
                    rd2 = dramp.tile([2, NT], F32, tag="rd2")
                    nc.sync.dma_start(
                        out=rd2[:].rearrange("a b -> (a b)").rearrange(
                            "(p j) -> p j", p=128
                        ),
                        in_=rseg,
                    )
                    for j in range(2):
                        rbc = rbp.tile([128, NT], F32, tag="rbc", bufs=2)
                        nc.scalar.dma_start(
                            out=rbc, in_=rd2[j : j + 1, :].to_broadcast([128, NT])
                        )
                        nc.vector.tensor_mul(
                            outn_sb[:, h, ds(i0 + j * NT, NT)],
                            osb[:, ts(j, NT)],
                            rbc,
                        )

            # ---- out projection (feature-major partial): outT[dout, s] ----
            out_q = [nc.sync, nc.scalar, nc.gpsimd]
            for it in range(IT):
                for dc in range(DOUT // 128):
                    facc = psp.tile([128, NT], F32, tag="acc")
                    for h in range(NH):
                        nc.tensor.matmul(
                            facc,
                            wo_sb[:, h, ds(dc * 128, 128)],
                            outn_sb[:, h, ts(it, NT)],
                            start=(h == 0),
                            stop=(h == NH - 1),
                        )
                    fsb = fop.tile([128, NT], F32, tag="f")
                    nc.scalar.activation(
                        out=fsb,
                        in_=facc,
                        func=mybir.ActivationFunctionType.Identity,
                        bias=bo_sb[:, dc : dc + 1],
                    )
                    out_q[dc % 3].dma_start(
                        out=outT[dc * 128 : (dc + 1) * 128, ts(it, NT)], in_=fsb
                    )

    return nc


_NC_CACHE = None


def _get_nc():
    global _NC_CACHE
    if _NC_CACHE is None:
        nc = build()
        nc.compile()
        _NC_CACHE = nc
    return _NC_CACHE


def make_in_maps(q, k, v, mask, Wq, bq, Wk, bk, Wv, bv, Wo, bo):
    bf = ml_dtypes.bfloat16
    q = np.asarray(q, np.float32)
    k = np.asarray(k, np.float32)
    v = np.asarray(v, np.float32)
    mask = np.asarray(mask)
    Ws = [np.asarray(w, np.float32) for w in (Wq, Wk, Wv)]
    bs = [np.asarray(b_, np.float32) for b_ in (bq, bk, bv)]
    Wo = np.asarray(Wo, np.float32)
    bo = np.asarray(bo, np.float32)

    xTb, maskTb = [], []
    for b in range(B):
        xTb.append(
            np.ascontiguousarray(np.stack([q[b].T, k[b].T, v[b].T]).astype(bf))
        )
        maskTb.append(
            np.ascontiguousarray(mask[b].T.astype(np.float32)).astype(bf)
        )
    # W[dm, dout] with head h owning columns d*H+h; reshape for tile slicing:
    # Wr[t][kc, p, d, h] = W[kc*128+p, d*H+h]
    Wr = [W.reshape(KC, 128, D, H) for W in Ws]
    br = [b_.reshape(D, H) for b_ in bs]

    in_maps = []
    for c in range(8):
        b = c // 4
        h0 = NH * (c % 4)
        w_core = np.empty((128, 3, KC, NH, D), np.float32)
        for t in range(3):
            for hi in range(NH):
                w_core[:, t, :, hi, :] = Wr[t][:, :, :, h0 + hi].transpose(1, 0, 2)
        # q/k biases as per-partition (d) columns; v bias as a bf16 row
        b_core = np.empty((128, 3, NH), np.float32)
        for t in range(3):
            for hi in range(NH):
                b_core[:, t, hi] = br[t][:, h0 + hi]
        bv_core = np.stack([br[2][:, h0 + hi] for hi in range(NH)])[None]
        wo_core = np.stack([Wo[h0 + hi :: H, :] for hi in range(NH)], axis=1)
        bo_core = bo if c % 4 == 0 else np.zeros_like(bo)
        in_maps.append(
            {
                "xT": xTb[b],
                "maskT": maskTb[b],
                "w_qkv": np.ascontiguousarray(w_core).astype(bf),
                "b_qkv": np.ascontiguousarray(b_core),
                "bv": np.ascontiguousarray(bv_core).astype(bf),
                "wo": np.ascontiguousarray(wo_core).astype(bf),
                "bo": np.ascontiguousarray(bo_core.reshape(DOUT // 128, 128).T),
            }
        )
    return in_maps


def unshard(results):
    out = np.zeros((B, DOUT, S), np.float32)
    for c in range(8):
        out[c // 4] += np.asarray(results[c]["outT"], np.float32)
    return np.ascontiguousarray(out.transpose(0, 2, 1))


def kernel(**inputs):
    in_maps = make_in_maps(**inputs)
    nc = _get_nc()
    res = run_bass_kernel_spmd(nc, in_maps, core_ids=list(range(8)))
    return unshard(res.results)


# revision 9
# speedup vs baseline: 1.1915x; 1.1915x over previous
"""Trainium2 Bass kernel for nn_MultiHeadAttention (B=2, S=2048, DM=1024, H=8).

Sharding: data-parallel on batch x tensor-parallel on heads.
Core c in 0..7 handles batch b = c//4 and heads {2*(c%4), 2*(c%4)+1}.
Each core computes its two heads' full attention and the partial
out-projection (a 1024x2048 partial sum); the host adds the 4 partials
per batch and transposes back to (S, DOUT).

Device dataflow is feature-major ("transposed") throughout:
  - host ships q/k/v transposed to (DM, S) per batch (bf16)
  - QpT/KpT (d, S) = W_h^T @ xT via matmuls with W stationary
  - Vp natural (S, d) computed directly with xT tiles stationary
  - S^T pair-tile (128 keys x 1024 queries, 2 PSUM banks) -> one ACT exp
    (scale=1/sqrt(D)) -> one DVE mask multiply (bf16 2x mode)
  - out^T accumulated over key chunks; rowsums via ones-column matmuls
    bank-packed at partitions 0/32 of one PSUM bank
  - early PSUM evacuation, reciprocal on a [128,8]-reshaped layout,
    DMA-broadcast, TT normalize
  - out-projection stays feature-major; bo added during DVE evacuation
"""

import sys

sys.path.insert(0, "/opt/trn_rl_repo")

import numpy as np
import ml_dtypes

import concourse.bass as bass
import concourse.tile as tile
from concourse import bacc, mybir
from concourse.bass import ts, ds
from concourse.bass_utils import run_bass_kernel_spmd

BF16 = mybir.dt.bfloat16
F32 = mybir.dt.float32
Copy = mybir.ActivationFunctionType.Copy
Exp = mybir.ActivationFunctionType.Exp
ADD = mybir.AluOpType.add
MULT = mybir.AluOpType.mult

B, S, DM, H, DOUT = 2, 2048, 1024, 8, 1024
D = DM // H            # 128 head dim
NH = 2                 # heads per core
KC = DM // 128         # 8 contraction chunks for projections
OC = S // 128          # 16 key chunks
NT = 512               # PSUM-bank-sized free tile (fp32)
IT = S // NT           # 4 query tiles
SCALE = float(1.0 / np.sqrt(np.float32(D)))


def build():
    nc = bacc.Bacc(None, target_bir_lowering=False)

    xT = nc.dram_tensor("xT", [3, DM, S], BF16, kind="ExternalInput")
    maskT = nc.dram_tensor("maskT", [S, S], BF16, kind="ExternalInput")
    w_qkv = nc.dram_tensor("w_qkv", [128, 3, KC, NH, D], BF16, kind="ExternalInput")
    b_qkv = nc.dram_tensor("b_qkv", [128, 3, NH], F32, kind="ExternalInput")
    bv_in = nc.dram_tensor("bv", [1, NH, D], BF16, kind="ExternalInput")
    wo = nc.dram_tensor("wo", [D, NH, DOUT], BF16, kind="ExternalInput")
    bo = nc.dram_tensor("bo", [128, DOUT // 128], F32, kind="ExternalInput")
    outT = nc.dram_tensor("outT", [DOUT, S], F32, kind="ExternalOutput")

    dma_q = [nc.sync, nc.scalar, nc.gpsimd]  # spread big loads over queues

    with tile.TileContext(nc) as tc:
        with (
            tc.tile_pool(name="const", bufs=1) as constp,
            tc.tile_pool(name="xin", bufs=9) as xp,
            tc.tile_pool(name="ptile", bufs=5) as pp,
            tc.tile_pool(name="rb", bufs=2) as rbp,
            tc.tile_pool(name="fout", bufs=4) as fop,
            tc.tile_pool(name="psum", bufs=4, space="PSUM") as psp,
            tc.tile_pool(name="dram", bufs=2, space="DRAM") as dramp,
        ):
            # ---- constants (small, load first) ----
            b_sb = constp.tile([128, 3, NH], F32)  # per-partition-d biases q/k
            nc.sync.dma_start(out=b_sb, in_=b_qkv[:])
            bv_sb = constp.tile([1, NH, D], BF16)
            nc.scalar.dma_start(out=bv_sb, in_=bv_in[:])
            bo_sb = constp.tile([128, DOUT // 128], F32)
            nc.gpsimd.dma_start(out=bo_sb, in_=bo[:])
            ones_row = constp.tile([1, NT], BF16)
            nc.vector.memset(ones_row, 1.0)
            ones_col = constp.tile([128, 1], BF16)
            nc.vector.memset(ones_col, 1.0)

            # ---- weight chunks + x tiles interleaved in need order ----
            w_sb = constp.tile([128, 3, KC, NH, D], BF16)
            wo_sb = constp.tile([D, NH, DOUT], BF16)
            xts_all = []
            for t in range(3):
                row = []
                for k in range(KC):
                    qi = t * KC + k
                    nc.sync.dma_start(
                        out=w_sb[:, t, k, :, :], in_=w_qkv[:, t, k, :, :]
                    )
                    xt = xp.tile([128, S], BF16, tag="x", name=f"x{t}_{k}")
                    dma_q[qi % 3].dma_start(
                        out=xt, in_=xT[t, k * 128 : (k + 1) * 128, :]
                    )
                    row.append(xt)
                xts_all.append(row)

            # mask after x (needed only from attention onward), all queues
            mask_sb = constp.tile([128, OC, S], BF16)
            for oc in range(OC):
                dma_q[oc % 3].dma_start(
                    out=mask_sb[:, oc, :], in_=maskT[oc * 128 : (oc + 1) * 128, :]
                )

            nc.scalar.dma_start(out=wo_sb, in_=wo[:])

            # ---- Q/K projections: qk_sb[d, t, h, s]; bias added on DVE evac ----
            qk_sb = constp.tile([128, 2, NH, S], BF16)
            for t in range(2):
                xts = xts_all[t]
                for h in range(NH):
                    acc = [
                        psp.tile([128, NT], F32, tag="acc", name=f"acc{it}")
                        for it in range(IT)
                    ]
                    for k in range(KC):
                        for it in range(IT):
                            nc.tensor.matmul(
                                acc[it],
                                w_sb[:, t, k, h, :],
                                xts[k][:, ts(it, NT)],
                                start=(k == 0),
                                stop=(k == KC - 1),
                            )
                    for it in range(IT):
                        nc.vector.tensor_scalar_add(
                            out=qk_sb[:, t, h, ts(it, NT)],
                            in0=acc[it],
                            scalar1=b_sb[:, t, h : h + 1],
                        )

            # ---- V projection, natural layout: vp_sb[s%128, oc, h, d] ----
            vp_sb = constp.tile([128, OC, NH, D], BF16)
            xts = xts_all[2]
            for oc in range(OC):
                vacc = psp.tile([128, NH * D], F32, tag="acc")
                nc.tensor.matmul(
                    vacc, ones_row[:, 0:128], bv_sb[:, :, :], start=True, stop=False
                )
                for k in range(KC):
                    nc.tensor.matmul(
                        vacc,
                        xts[k][:, ts(oc, 128)],
                        w_sb[:, 2, k, :, :],
                        start=False,
                        stop=(k == KC - 1),
                    )
                nc.scalar.activation(out=vp_sb[:, oc, :, :], in_=vacc, func=Copy)

            # ---- attention per head / per half (1024 queries) ----
            outn_sb = constp.tile([128, NH, S], BF16)
            for half in range(2):
                for h in range(NH):
                    i0 = half * 2 * NT  # start query of this half
                    outp = [
                        psp.tile([128, NT], F32, tag="acc", name=f"outp{j}")
                        for j in range(2)
                    ]
                    rp = psp.tile([128, NT], F32, tag="acc", name="rp")
                    for oc in range(OC):
                        pms = []
                        for j in range(2):
                            sps = psp.tile(
                                [128, NT], F32, tag="s", bufs=4, name=f"sps{j}"
                            )
                            nc.tensor.matmul(
                                sps,
                                qk_sb[:, 1, h, ds(oc * 128, 128)],
                                qk_sb[:, 0, h, ds(i0 + j * NT, NT)],
                                start=True,
                                stop=True,
                            )
                            p = pp.tile([128, NT], BF16, tag="p", name=f"p{j}")
                            nc.scalar.activation(
                                out=p, in_=sps, func=Exp, bias=0.0, scale=SCALE
                            )
                            pm = pp.tile([128, NT], BF16, tag="pm", name=f"pm{j}")
                            nc.vector.tensor_mul(
                                pm, p, mask_sb[:, oc, ds(i0 + j * NT, NT)]
                            )
                            pms.append(pm)
                        for j in range(2):
                            nc.tensor.matmul(
                                outp[j],
                                vp_sb[:, oc, h, :],
                                pms[j],
                                start=(oc == 0),
                                stop=(oc == OC - 1),
                            )
                            nc.tensor.matmul(
                                rp[32 * j : 32 * j + 1, :],
                                ones_col,
                                pms[j],
                                start=(oc == 0),
                                stop=(oc == OC - 1),
                            )
                    # early evacuation frees the PSUM banks for the next half
                    osb = rbp.tile([128, 2 * NT], F32, tag="osb")
                    for j in range(2):
                        nc.vector.tensor_copy(osb[:, ts(j, NT)], outp[j])
                    r2 = rbp.tile([33, NT], F32, tag="r2")
                    for j in range(2):
                        nc.vector.tensor_copy(
                            r2[32 * j : 32 * j + 1, :], rp[32 * j : 32 * j + 1, :]
                        )
                    # reshape rowsums through DRAM so reciprocal uses 128 lanes
                    rd = dramp.tile([2, NT], F32, tag="rd")
                    for j in range(2):
                        nc.sync.dma_start(
                            out=rd[j : j + 1, :], in_=r2[32 * j : 32 * j + 1, :]
                        )
                    rseg = rbp.tile([128, 8], F32, tag="rseg")
                    nc.sync.dma_start(
                        out=rseg,
                        in_=rd[:].rearrange("a b -> (a b)").rearrange(
                            "(p j) -> p j", p=128
                        ),
                    )
                    nc.vector.reciprocal(rseg, rseg)
                    rd2 = dramp.tile([2, NT], F32, tag="rd2")
                    nc.sync.dma_start(
                        out=rd2[:].rearrange("a b -> (a b)").rearrange(
                            "(p j) -> p j", p=128
                        ),
                        in_=rseg,
                    )
                    for j in range(2):
                        rbc = rbp.tile([128, NT], F32, tag="rbc", bufs=2)
                        nc.gpsimd.dma_start(
                            out=rbc, in_=rd2[j : j + 1, :].to_broadcast([128, NT])
                        )
                        nc.vector.tensor_mul(
                            outn_sb[:, h, ds(i0 + j * NT, NT)],
                            osb[:, ts(j, NT)],
                            rbc,
                        )

            # ---- out projection (feature-major partial): outT[dout, s] ----
            out_q = [nc.sync, nc.scalar, nc.gpsimd]
            for it in range(IT):
                for dc in range(DOUT // 128):
                    facc = psp.tile([128, NT], F32, tag="acc")
                    for h in range(NH):
                        nc.tensor.matmul(
                            facc,
                            wo_sb[:, h, ds(dc * 128, 128)],
                            outn_sb[:, h, ts(it, NT)],
                            start=(h == 0),
                            stop=(h == NH - 1),
                        )
                    fsb = fop.tile([128, NT], F32, tag="f")
                    nc.vector.tensor_scalar_add(
                        out=fsb, in0=facc, scalar1=bo_sb[:, dc : dc + 1]
                    )
                    out_q[dc % 3].dma_start(
                        out=outT[dc * 128 : (dc + 1) * 128, ts(it, NT)], in_=fsb
                    )

    return nc


_NC_CACHE = None


def _get_nc():
    global _NC_CACHE
    if _NC_CACHE is None:
        nc = build()
        nc.compile()
        _NC_CACHE = nc
    return _NC_CACHE


def make_in_maps(q, k, v, mask, Wq, bq, Wk, bk, Wv, bv, Wo, bo):
    bf = ml_dtypes.bfloat16
    q = np.asarray(q, np.float32)
    k = np.asarray(k, np.float32)
    v = np.asarray(v, np.float32)
    mask = np.asarray(mask)
    Ws = [np.asarray(w, np.float32) for w in (Wq, Wk, Wv)]
    bs = [np.asarray(b_, np.float32) for b_ in (bq, bk, bv)]
    Wo = np.asarray(Wo, np.float32)
    bo = np.asarray(bo, np.float32)

    xTb, maskTb = [], []
    for b in range(B):
        xTb.append(
            np.ascontiguousarray(np.stack([q[b].T, k[b].T, v[b].T]).astype(bf))
        )
        maskTb.append(
            np.ascontiguousarray(mask[b].T.astype(np.float32)).astype(bf)
        )
    # W[dm, dout] with head h owning columns d*H+h; reshape for tile slicing:
    # Wr[t][kc, p, d, h] = W[kc*128+p, d*H+h]
    Wr = [W.reshape(KC, 128, D, H) for W in Ws]
    br = [b_.reshape(D, H) for b_ in bs]

    in_maps = []
    for c in range(8):
        b = c // 4
        h0 = NH * (c % 4)
        w_core = np.empty((128, 3, KC, NH, D), np.float32)
        for t in range(3):
            for hi in range(NH):
                w_core[:, t, :, hi, :] = Wr[t][:, :, :, h0 + hi].transpose(1, 0, 2)
        # q/k biases as per-partition (d) columns; v bias as a bf16 row
        b_core = np.empty((128, 3, NH), np.float32)
        for t in range(3):
            for hi in range(NH):
                b_core[:, t, hi] = br[t][:, h0 + hi]
        bv_core = np.stack([br[2][:, h0 + hi] for hi in range(NH)])[None]
        wo_core = np.stack([Wo[h0 + hi :: H, :] for hi in range(NH)], axis=1)
        bo_core = bo if c % 4 == 0 else np.zeros_like(bo)
        in_maps.append(
            {
                "xT": xTb[b],
                "maskT": maskTb[b],
                "w_qkv": np.ascontiguousarray(w_core).astype(bf),
                "b_qkv": np.ascontiguousarray(b_core),
                "bv": np.ascontiguousarray(bv_core).astype(bf),
                "wo": np.ascontiguousarray(wo_core).astype(bf),
                "bo": np.ascontiguousarray(bo_core.reshape(DOUT // 128, 128).T),
            }
        )
    return in_maps


def unshard(results):
    out = np.zeros((B, DOUT, S), np.float32)
    for c in range(8):
        out[c // 4] += np.asarray(results[c]["outT"], np.float32)
    return np.ascontiguousarray(out.transpose(0, 2, 1))


def kernel(**inputs):
    in_maps = make_in_maps(**inputs)
    nc = _get_nc()
    res = run_bass_kernel_spmd(nc, in_maps, core_ids=list(range(8)))
    return unshard(res.results)
